# revision 1
# baseline (speedup 1.0000x reference)
"""Causal self-attention (token-shift + QK-RMSNorm + RoPE + value-residual)
Trainium2 Bass kernel, sharded over 8 NeuronCores.

Sharding: core c handles batch b=c//4 and head-group g=c%4 (4 heads, 512
channels). Each core computes q/k/v projections for its channels, attention
for its heads, and a partial c_proj (its 512 input rows of Wproj). Host sums
the 4 partials per batch and adds the residual.

All matmuls are bf16 x bf16 with fp32 PSUM accumulation. Device layout is
transposed ([channel, time]) so the contraction dim always sits on SBUF
partitions; scores are computed as S^T = K^T_tile.T-style matmuls, softmax
uses a ones-matmul partition-broadcast row-sum (no max subtraction needed:
|scores| <= sqrt(D) after RMS norm), and RoPE's half-swap goes through a
SBUF->SBUF DMA (elementwise engines cannot cross partition bases).
"""
import sys

sys.path.insert(0, "/opt/trn_rl_repo")

import numpy as np
import ml_dtypes

B, T, C, H, D = 2, 2048, 2048, 16, 128
NCORES = 8
LC = 512          # local channels per core (4 heads)
TQ = 512          # tq chunk size
NKT = C // 128    # 16 k-tiles over the C contraction
NCHUNK = T // TQ  # 4
ROPE_THETA = 10000.0
MASK_NEG = -1.0e5
EPS = float(np.finfo(np.float32).eps)

_bf = ml_dtypes.bfloat16

_prog_cache = {}


def _build_program():
    import concourse.bass as bass
    import concourse.mybir as mybir
    from concourse import bacc
    from concourse.tile import TileContext
    from concourse.alu_op_type import AluOpType

    AFt = mybir.ActivationFunctionType
    if not getattr(bacc, "_act_tables_pinned", False):
        _orig_gat = bacc.get_activation_tables

        def _pinned_gat(arch):
            tables = _orig_gat(arch)
            pinned = {AFt.Ln, AFt.Exp, AFt.Square}
            for name, fns in tables.items():
                if name != "natural_log_exp_and_others":
                    fns -= pinned
            return tables

        bacc.get_activation_tables = _pinned_gat
        bacc._act_tables_pinned = True

    F32 = mybir.dt.float32
    BF16 = mybir.dt.bfloat16
    AF = mybir.ActivationFunctionType

    nc = bacc.Bacc("TRN2", target_bir_lowering=False, debug=False)

    xbT = nc.dram_tensor("xbT", [C, T], BF16, kind="ExternalInput").ap()
    wq = nc.dram_tensor("wq", [C, LC], BF16, kind="ExternalInput").ap()
    wk = nc.dram_tensor("wk", [C, LC], BF16, kind="ExternalInput").ap()
    wv = nc.dram_tensor("wv", [C, LC], BF16, kind="ExternalInput").ap()
    wp = nc.dram_tensor("wp", [LC, C], BF16, kind="ExternalInput").ap()
    v1l = nc.dram_tensor("v1l", [T, LC], BF16, kind="ExternalInput").ap()
    cos2 = nc.dram_tensor("cos2", [128, T], BF16, kind="ExternalInput").ap()
    sin2 = nc.dram_tensor("sin2", [128, T], BF16, kind="ExternalInput").ap()
    masks = nc.dram_tensor("masks", [128, 896], F32, kind="ExternalInput").ap()
    perm = nc.dram_tensor("perm", [128, 128], BF16, kind="ExternalInput").ap()
    outT = nc.dram_tensor("outT", [C, T], F32, kind="ExternalOutput").ap()

    SCALE = 1.0 / float(np.sqrt(D))

    with TileContext(nc) as tc:
        with (
            tc.tile_pool(name="cpool", bufs=1) as cpool,
            tc.tile_pool(name="kvpool", bufs=1) as kvpool,
            tc.tile_pool(name="xpool", bufs=2) as xpool,
            tc.tile_pool(name="qpool", bufs=2) as qpool,
            tc.tile_pool(name="apool", bufs=2) as apool,
            tc.tile_pool(name="epool", bufs=4) as epool,
            tc.tile_pool(name="wpool", bufs=2) as wpool,
            tc.tile_pool(name="opool", bufs=3) as opool,
            tc.tile_pool(name="pspool", bufs=1, space="PSUM") as pspool,
        ):
            # ---- constant loads (before barrier) ----
            wq_t = []
            wk_t = []
            wv_t = []
            for kt in range(NKT):
                t_ = cpool.tile([128, LC], BF16, tag=f"wq{kt}", name=f"wq{kt}")
                nc.sync.dma_start(t_, wq[128 * kt:128 * kt + 128, :])
                wq_t.append(t_)
                t_ = cpool.tile([128, LC], BF16, tag=f"wk{kt}", name=f"wk{kt}")
                nc.sync.dma_start(t_, wk[128 * kt:128 * kt + 128, :])
                wk_t.append(t_)
                t_ = cpool.tile([128, LC], BF16, tag=f"wv{kt}", name=f"wv{kt}")
                nc.sync.dma_start(t_, wv[128 * kt:128 * kt + 128, :])
                wv_t.append(t_)
            wp_t = []
            for m in range(4):
                t_ = cpool.tile([128, C], BF16, tag=f"wp{m}", name=f"wp{m}")
                nc.sync.dma_start(t_, wp[128 * m:128 * m + 128, :])
                wp_t.append(t_)
            cos_sb = cpool.tile([128, T], BF16, tag="cos", name="cos_sb")
            nc.sync.dma_start(cos_sb, cos2)
            sin_sb = cpool.tile([128, T], BF16, tag="sin", name="sin_sb")
            nc.sync.dma_start(sin_sb, sin2)
            mask_sb = cpool.tile([128, 896], F32, tag="mask", name="mask_sb")
            nc.sync.dma_start(mask_sb, masks)
            ones = cpool.tile([128, 128], BF16, tag="ones", name="ones")
            nc.vector.memset(ones, 1.0)
            epst = cpool.tile([128, 1], F32, tag="epst", name="epst")
            nc.vector.memset(epst, EPS)
            perm_sb = cpool.tile([128, 128], BF16, tag="perm", name="perm_sb")
            nc.sync.dma_start(perm_sb, perm)

            # persistent K^T (per head) and V stores
            kT = [
                kvpool.tile([128, T], BF16, tag=f"kT{m}", name=f"kT{m}")
                for m in range(4)
            ]
            vst = [
                kvpool.tile([128, LC], BF16, tag=f"v{i}", name=f"v{i}")
                for i in range(T // 128)
            ]

            for j in range(NCHUNK):
                tq0 = TQ * j
                # ---- load xb chunk ----
                xb_t = []
                for kt in range(NKT):
                    t_ = xpool.tile([128, TQ], BF16, tag=f"xb{kt}", name=f"xb{kt}_{j}")
                    nc.sync.dma_start(t_, xbT[128 * kt:128 * kt + 128, tq0:tq0 + TQ])
                    xb_t.append(t_)

                # ---- q/k projections + rmsnorm + rope ----
                qT = []
                for which, wt in (("q", wq_t), ("k", wk_t)):
                    for m in range(4):
                        q_ps = pspool.tile([128, TQ], F32, tag="mm", bufs=2,
                                           name=f"{which}ps{m}_{j}")
                        for kt in range(NKT):
                            nc.tensor.matmul(
                                q_ps,
                                wt[kt][:, 128 * m:128 * m + 128],
                                xb_t[kt],
                                start=(kt == 0),
                                stop=(kt == NKT - 1),
                            )
                        q_sb = wpool.tile([128, TQ], BF16, tag="qsb",
                                          name=f"{which}sb{m}_{j}")
                        nc.vector.tensor_copy(q_sb, q_ps)
                        sq = wpool.tile([128, TQ], BF16, tag="sq",
                                        name=f"{which}sq{m}_{j}")
                        nc.vector.tensor_mul(sq, q_sb, q_sb)
                        ss_ps = pspool.tile([128, TQ], F32, tag="ss", bufs=1,
                                            name=f"{which}ss{m}_{j}")
                        nc.tensor.matmul(ss_ps, ones, sq, start=True, stop=True)
                        lnt = wpool.tile([128, TQ], F32, tag="lnt",
                                         name=f"{which}ln{m}_{j}")
                        nc.scalar.activation(lnt, ss_ps, AF.Ln,
                                             scale=1.0 / D, bias=epst)
                        rms = wpool.tile([128, TQ], BF16, tag="rms",
                                         name=f"{which}rms{m}_{j}")
                        nc.scalar.activation(rms, lnt, AF.Exp, scale=-0.5)
                        sw_ps = pspool.tile([128, TQ], F32, tag="swp", bufs=1,
                                            name=f"{which}swp{m}_{j}")
                        nc.tensor.matmul(sw_ps, perm_sb, q_sb,
                                         start=True, stop=True)
                        t1 = wpool.tile([128, TQ], BF16, tag="t1",
                                        name=f"{which}t1{m}_{j}")
                        nc.vector.tensor_mul(t1, q_sb, cos_sb[:, tq0:tq0 + TQ])
                        t2 = wpool.tile([128, TQ], BF16, tag="t2",
                                        name=f"{which}t2{m}_{j}")
                        nc.vector.tensor_mul(t2, sw_ps, sin_sb[:, tq0:tq0 + TQ])
                        t3 = wpool.tile([128, TQ], BF16, tag="t3",
                                        name=f"{which}t3{m}_{j}")
                        nc.vector.tensor_add(t3, t1, t2)
                        if which == "q":
                            dst = qpool.tile([128, TQ], BF16, tag=f"qT{m}",
                                             name=f"qT{m}_{j}")
                            nc.vector.tensor_mul(dst, t3, rms)
                            qT.append(dst)
                        else:
                            nc.vector.tensor_mul(kT[m][:, tq0:tq0 + TQ], t3, rms)

                # ---- v projection + value-residual blend ----
                for tt in range(4):
                    v_ps = pspool.tile([128, LC], F32, tag="mm", bufs=2,
                                       name=f"vps{tt}_{j}")
                    for kt in range(NKT):
                        nc.tensor.matmul(
                            v_ps,
                            xb_t[kt][:, 128 * tt:128 * tt + 128],
                            wv_t[kt],
                            start=(kt == 0),
                            stop=(kt == NKT - 1),
                        )
                    v1t = wpool.tile([128, LC], BF16, tag="v1t",
                                     name=f"v1t{tt}_{j}")
                    nc.sync.dma_start(
                        v1t, v1l[tq0 + 128 * tt:tq0 + 128 * tt + 128, :])
                    nc.vector.tensor_add(vst[4 * j + tt], v_ps, v1t)

                # ---- attention per head ----
                attnT = []
                ntk = 4 * (j + 1)
                for h in range(4):
                    pv_ps = pspool.tile([128, TQ], F32, tag="pv", bufs=1,
                                        name=f"pv{h}_{j}")
                    se_ps = pspool.tile([128, TQ], F32, tag="se", bufs=1,
                                        name=f"se{h}_{j}")
                    for tk in range(ntk):
                        s_ps = pspool.tile([128, TQ], F32, tag="s", bufs=2,
                                           name=f"s{h}_{tk}_{j}")
                        nc.tensor.matmul(
                            s_ps,
                            kT[h][:, 128 * tk:128 * tk + 128],
                            qT[h],
                            start=True,
                            stop=True,
                        )
                        if tk >= 4 * j:  # diagonal tile: causal mask add
                            d_ = 128 * tk - tq0
                            s0 = 384 - d_
                            nc.vector.tensor_add(
                                s_ps, s_ps, mask_sb[:, s0:s0 + TQ])
                        e_t = epool.tile([128, TQ], BF16, tag="e",
                                         name=f"e{h}_{tk}_{j}")
                        nc.scalar.activation(e_t, s_ps, AF.Exp, scale=SCALE)
                        nc.tensor.matmul(
                            pv_ps,
                            vst[tk][:, 128 * h:128 * h + 128],
                            e_t,
                            start=(tk == 0),
                            stop=(tk == ntk - 1),
                        )
                        nc.tensor.matmul(
                            se_ps, ones, e_t,
                            start=(tk == 0), stop=(tk == ntk - 1),
                        )
                    lnse = wpool.tile([128, TQ], F32, tag="lnse",
                                      name=f"lnse{h}_{j}")
                    nc.scalar.activation(lnse, se_ps, AF.Ln)
                    rec = wpool.tile([128, TQ], BF16, tag="rec",
                                     name=f"rec{h}_{j}")
                    nc.scalar.activation(rec, lnse, AF.Exp, scale=-1.0)
                    at = apool.tile([128, TQ], BF16, tag=f"attn{h}",
                                    name=f"attn{h}_{j}")
                    nc.vector.tensor_mul(at, pv_ps, rec)
                    attnT.append(at)

                # ---- partial c_proj ----
                for co in range(16):
                    o_ps = pspool.tile([128, TQ], F32, tag="mm", bufs=2,
                                       name=f"ops{co}_{j}")
                    for m in range(4):
                        nc.tensor.matmul(
                            o_ps,
                            wp_t[m][:, 128 * co:128 * co + 128],
                            attnT[m],
                            start=(m == 0),
                            stop=(m == 3),
                        )
                    o_sb = opool.tile([128, TQ], F32, tag="osb",
                                      name=f"osb{co}_{j}")
                    nc.vector.tensor_copy(o_sb, o_ps)
                    nc.sync.dma_start(
                        outT[128 * co:128 * co + 128, tq0:tq0 + TQ], o_sb)

    nc.finalize()
    return nc


def _host_prep(inputs):
    """Build the 8 per-core input maps (all numpy)."""
    x = np.asarray(inputs["x"], np.float32)
    v1 = np.asarray(inputs["v1"], np.float32)
    x_q = np.asarray(inputs["x_q"], np.float32)
    x_k = np.asarray(inputs["x_k"], np.float32)
    x_v = np.asarray(inputs["x_v"], np.float32)
    Wq = np.asarray(inputs["Wq"], np.float32)
    Wk = np.asarray(inputs["Wk"], np.float32)
    Wv = np.asarray(inputs["Wv"], np.float32)
    Wproj = np.asarray(inputs["Wproj"], np.float32)
    lamb = float(np.asarray(inputs["lamb"]))

    assert np.array_equal(x_q, x_k) and np.array_equal(x_q, x_v), (
        "kernel assumes shared token-shift mix vectors (x_q == x_k == x_v)"
    )

    # token-shift blend, then transpose per batch
    sh = np.concatenate([np.zeros((B, 1, C), np.float32), x[:, :-1]], axis=1)
    xb = x * (1.0 - x_q) + sh * x_q
    xbT = [np.ascontiguousarray(xb[b_].T).astype(_bf) for b_ in range(B)]

    # rope tables, duplicated halves; sin second half negated
    inv = 1.0 / (ROPE_THETA ** (np.arange(0, D, 2, dtype=np.float32) / D))
    fr = np.outer(np.arange(T, dtype=np.float32), inv)  # [T, 64]
    cosT = np.cos(fr).T.astype(np.float32)  # [64, T]
    sinT = np.sin(fr).T.astype(np.float32)
    cos2 = np.concatenate([cosT, cosT], axis=0).astype(_bf)
    sin2 = np.concatenate([sinT, -sinT], axis=0).astype(_bf)

    # causal mask master strip: M[p, g] = 0 if g >= p + 384 else MASK_NEG
    p = np.arange(128)[:, None]
    g = np.arange(896)[None, :]
    masks = np.where(g >= p + 384, 0.0, MASK_NEG).astype(np.float32)
    permm = np.roll(np.eye(128, dtype=np.float32), 64, axis=0).astype(_bf)

    in_maps = []
    for c in range(NCORES):
        b_ = c // 4
        g_ = c % 4
        L = slice(LC * g_, LC * g_ + LC)
        in_maps.append({
            "xbT": xbT[b_],
            "wq": np.ascontiguousarray(Wq[L].T).astype(_bf),
            "wk": np.ascontiguousarray(Wk[L].T).astype(_bf),
            "wv": np.ascontiguousarray(((1.0 - lamb) * Wv[L]).T).astype(_bf),
            "wp": np.ascontiguousarray(Wproj[:, L].T).astype(_bf),
            "v1l": (lamb * v1[b_][:, L]).astype(_bf),
            "cos2": cos2,
            "sin2": sin2,
            "masks": masks,
            "perm": permm,
        })
    return in_maps


def _run(in_maps, trace=False):
    from concourse.bass_utils import run_bass_kernel_spmd

    if "nc" not in _prog_cache:
        _prog_cache["nc"] = _build_program()
    return run_bass_kernel_spmd(
        _prog_cache["nc"], in_maps, core_ids=list(range(NCORES)), trace=trace
    )


def kernel(**inputs) -> np.ndarray:
    residual = np.asarray(inputs["residual"], np.float32)
    in_maps = _host_prep(inputs)
    res = _run(in_maps)
    out = np.empty((B, T, C), np.float32)
    for b_ in range(B):
        acc = res.results[4 * b_]["outT"].astype(np.float32)
        for g_ in range(1, 4):
            acc = acc + res.results[4 * b_ + g_]["outT"]
        out[b_] = residual[b_] + acc.T
    return out



# revision 3
# speedup vs baseline: 1.7790x; 1.7790x over previous
"""Causal self-attention (token-shift + QK-RMSNorm + RoPE + value-residual)
Trainium2 Bass kernel, sharded over 8 NeuronCores.

Sharding: core c handles batch b=c//4 and head-group g=c%4 (4 heads, 512
channels). Each core computes q/k/v projections for its channels, attention
for its heads, and a partial c_proj (its 512 input rows of Wproj). Host sums
the 4 partials per batch and adds the residual.

v2: projections (q/k/v/out), the P@V contraction and the softmax denominator
run as fp8(e4m3) DoubleRow matmuls (two 128-deep k-subtiles contracted per
pass -> 2x tensor throughput). Scores stay bf16. Host pre-scales weights
(x32) and values (x16) so fp8 quantization error stays small; the rmsnorm
on q/k cancels the weight scale exactly, and the output partials carry a
4096x factor the host divides out. Diagonal score blocks are sliced to the
causally-valid column range; mask adds use narrow 128/256-wide strips.
"""
import sys

sys.path.insert(0, "/opt/trn_rl_repo")

import numpy as np
import ml_dtypes

B, T, C, H, D = 2, 2048, 2048, 16, 128
NCORES = 8
LC = 512          # local channels per core (4 heads)
TQ = 512          # tq chunk size
NKT = C // 128    # 16 k-tiles over the C contraction
NPAIR = NKT // 2  # 8 DoubleRow pairs
NCHUNK = T // TQ  # 4
ROPE_THETA = 10000.0
MASK_NEG = -1.0e5
EPS = float(np.finfo(np.float32).eps)
WSCALE = 32.0     # fp8 pre-scale on Wq/Wk/Wv/Wproj
VSCALE = 16.0     # fp8 pre-scale on v (folded into wv and v1)
ASCALE = 1.0      # attn tiles keep the 16x v-scale only (fp8 max is 240)
OUT_DIV = WSCALE * VSCALE * ASCALE  # 4096: host divides partials

_bf = ml_dtypes.bfloat16
_f8 = ml_dtypes.float8_e4m3

_prog_cache = {}


def _build_program():
    import concourse.bass as bass
    import concourse.mybir as mybir
    from concourse import bacc
    from concourse.tile import TileContext

    AFt = mybir.ActivationFunctionType
    if not getattr(bacc, "_act_tables_pinned", False):
        _orig_gat = bacc.get_activation_tables

        def _pinned_gat(arch):
            tables = _orig_gat(arch)
            pinned = {AFt.Ln, AFt.Exp, AFt.Square}
            for name, fns in tables.items():
                if name != "natural_log_exp_and_others":
                    fns -= pinned
            return tables

        bacc.get_activation_tables = _pinned_gat
        bacc._act_tables_pinned = True

    F32 = mybir.dt.float32
    BF16 = mybir.dt.bfloat16
    FP8 = mybir.dt.float8e4
    AF = mybir.ActivationFunctionType
    DR = mybir.MatmulPerfMode.DoubleRow

    nc = bacc.Bacc("TRN2", target_bir_lowering=False, debug=False)

    xbI = nc.dram_tensor("xbI", [128, NKT, T], FP8, kind="ExternalInput").ap()
    wqI = nc.dram_tensor("wqI", [128, NKT, LC], FP8, kind="ExternalInput").ap()
    wkI = nc.dram_tensor("wkI", [128, NKT, LC], FP8, kind="ExternalInput").ap()
    wvI = nc.dram_tensor("wvI", [128, NKT, LC], FP8, kind="ExternalInput").ap()
    wpI = nc.dram_tensor("wpI", [128, 4, C], FP8, kind="ExternalInput").ap()
    v1l = nc.dram_tensor("v1l", [T, LC], BF16, kind="ExternalInput").ap()
    cos2 = nc.dram_tensor("cos2", [128, T], BF16, kind="ExternalInput").ap()
    sin2 = nc.dram_tensor("sin2", [128, T], BF16, kind="ExternalInput").ap()
    mask256 = nc.dram_tensor("mask256", [128, 256], F32,
                             kind="ExternalInput").ap()
    perm = nc.dram_tensor("perm", [128, 128], BF16, kind="ExternalInput").ap()
    outT = nc.dram_tensor("outT", [C, T], F32, kind="ExternalOutput").ap()

    SCALE = 1.0 / float(np.sqrt(D))
    LN_A = float(np.log(ASCALE))

    with TileContext(nc) as tc:
        with (
            tc.tile_pool(name="cpool", bufs=1) as cpool,
            tc.tile_pool(name="kvpool", bufs=1) as kvpool,
            tc.tile_pool(name="xpool", bufs=2) as xpool,
            tc.tile_pool(name="qpool", bufs=2) as qpool,
            tc.tile_pool(name="apool", bufs=2) as apool,
            tc.tile_pool(name="epool", bufs=4) as epool,
            tc.tile_pool(name="wpool", bufs=2) as wpool,
            tc.tile_pool(name="opool", bufs=3) as opool,
            tc.tile_pool(name="pspool", bufs=1, space="PSUM") as pspool,
        ):
            # ---- constant loads, priority order for startup overlap ----
            wq_t = cpool.tile([128, NKT, LC], FP8, tag="wq", name="wq")
            nc.sync.dma_start(wq_t, wqI)
            xb_t0 = xpool.tile([128, NKT, TQ], FP8, tag="xb", name="xb_0")
            nc.sync.dma_start(xb_t0, xbI[:, :, 0:TQ])
            wk_t = cpool.tile([128, NKT, LC], FP8, tag="wk", name="wk")
            nc.sync.dma_start(wk_t, wkI)
            cos_sb = cpool.tile([128, T], BF16, tag="cos", name="cos_sb")
            nc.sync.dma_start(cos_sb, cos2)
            sin_sb = cpool.tile([128, T], BF16, tag="sin", name="sin_sb")
            nc.sync.dma_start(sin_sb, sin2)
            perm_sb = cpool.tile([128, 128], BF16, tag="perm", name="perm_sb")
            nc.sync.dma_start(perm_sb, perm)
            wv_t = cpool.tile([128, NKT, LC], FP8, tag="wv", name="wv")
            nc.sync.dma_start(wv_t, wvI)
            mask_sb = cpool.tile([128, 256], F32, tag="mask", name="mask_sb")
            nc.sync.dma_start(mask_sb, mask256)
            wp_t = cpool.tile([128, 4, C], FP8, tag="wp", name="wp")
            nc.sync.dma_start(wp_t, wpI)

            ones_bf = cpool.tile([128, 128], BF16, tag="onesb", name="ones_bf")
            nc.vector.memset(ones_bf, 1.0)
            ones8 = cpool.tile([128, 2, 128], FP8, tag="ones8", name="ones8")
            nc.vector.memset(ones8, 1.0)
            epst = cpool.tile([128, 1], F32, tag="epst", name="epst")
            nc.vector.memset(epst, EPS)

            # persistent K^T (per head) and paired-V stores
            kT = [
                kvpool.tile([128, T], BF16, tag=f"kT{m}", name=f"kT{m}")
                for m in range(4)
            ]
            vp = [
                kvpool.tile([128, 2, LC], FP8, tag=f"v{i}", name=f"v{i}")
                for i in range(T // 256)
            ]

            attn_prev = None  # [pair0, pair1] fp8 tiles of previous chunk

            def oproj(j, attn2, cos):
                """Issue 4 c_proj output tiles for chunk j from attn pair
                tiles; co = first output tile index."""
                tq0 = TQ * j
                for co in range(cos, cos + 4):
                    o_ps = pspool.tile([128, TQ], F32, tag="pa", bufs=2,
                                       name=f"ops{co}_{j}")
                    nc.tensor.matmul(
                        o_ps, wp_t[:, 0:2, 128 * co:128 * co + 128],
                        attn2[0], start=True, stop=False, perf_mode=DR)
                    nc.tensor.matmul(
                        o_ps, wp_t[:, 2:4, 128 * co:128 * co + 128],
                        attn2[1], start=False, stop=True, perf_mode=DR)
                    o_sb = opool.tile([128, TQ], F32, tag="osb",
                                      name=f"osb{co}_{j}")
                    nc.vector.tensor_copy(o_sb, o_ps)
                    nc.sync.dma_start(
                        outT[128 * co:128 * co + 128, tq0:tq0 + TQ], o_sb)

            for j in range(NCHUNK):
                tq0 = TQ * j
                # ---- load xb chunk ----
                if j == 0:
                    xb_t = xb_t0
                else:
                    xb_t = xpool.tile([128, NKT, TQ], FP8, tag="xb",
                                      name=f"xb_{j}")
                    nc.sync.dma_start(xb_t, xbI[:, :, tq0:tq0 + TQ])

                # ---- q/k projections + rmsnorm + rope ----
                qT = []
                for m in range(4):
                    for which, wt in (("q", wq_t), ("k", wk_t)):
                        q_ps = pspool.tile([128, TQ], F32, tag="pa", bufs=2,
                                           name=f"{which}ps{m}_{j}")
                        for g in range(NPAIR):
                            nc.tensor.matmul(
                                q_ps,
                                wt[:, 2 * g:2 * g + 2, 128 * m:128 * m + 128],
                                xb_t[:, 2 * g:2 * g + 2, :],
                                start=(g == 0),
                                stop=(g == NPAIR - 1),
                                perf_mode=DR,
                            )
                        q_sb = wpool.tile([128, TQ], BF16, tag="qsb",
                                          name=f"{which}sb{m}_{j}")
                        nc.scalar.activation(q_sb, q_ps, AF.Copy)
                        sq = wpool.tile([128, TQ], BF16, tag="sq",
                                        name=f"{which}sq{m}_{j}")
                        nc.scalar.activation(sq, q_sb, AF.Square)
                        ss_ps = pspool.tile([128, 2, TQ], F32, tag="s2",
                                            bufs=2, name=f"{which}ss{m}_{j}")
                        nc.tensor.matmul(ss_ps[:, 0, :], ones_bf, sq,
                                         start=True, stop=True)
                        lnt = wpool.tile([128, TQ], F32, tag="lnt",
                                         name=f"{which}ln{m}_{j}")
                        nc.scalar.activation(lnt, ss_ps[:, 0, :], AF.Ln,
                                             scale=1.0 / D, bias=epst)
                        rms = wpool.tile([128, TQ], BF16, tag="rms",
                                         name=f"{which}rms{m}_{j}")
                        nc.scalar.activation(rms, lnt, AF.Exp, scale=-0.5)
                        sw_ps = pspool.tile([128, 2, TQ], F32, tag="s2",
                                            bufs=2, name=f"{which}swp{m}_{j}")
                        nc.tensor.matmul(sw_ps[:, 0, :], perm_sb, q_sb,
                                         start=True, stop=True)
                        t1 = wpool.tile([128, TQ], BF16, tag="t1",
                                        name=f"{which}t1{m}_{j}")
                        nc.vector.tensor_mul(t1, q_sb, cos_sb[:, tq0:tq0 + TQ])
                        t2 = wpool.tile([128, TQ], BF16, tag="t2",
                                        name=f"{which}t2{m}_{j}")
                        nc.vector.tensor_mul(t2, sw_ps[:, 0, :],
                                             sin_sb[:, tq0:tq0 + TQ])
                        t3 = wpool.tile([128, TQ], BF16, tag="t3",
                                        name=f"{which}t3{m}_{j}")
                        nc.vector.tensor_add(t3, t1, t2)
                        if which == "q":
                            dst = qpool.tile([128, TQ], BF16, tag=f"qT{m}",
                                             name=f"qT{m}_{j}")
                            nc.vector.tensor_mul(dst, t3, rms)
                            qT.append(dst)
                        else:
                            nc.vector.tensor_mul(kT[m][:, tq0:tq0 + TQ],
                                                 t3, rms)

                # ---- v projection + value-residual blend (fp8 pairs) ----
                for tt in range(4):
                    v_ps = pspool.tile([128, LC], F32, tag="pa", bufs=2,
                                       name=f"vps{tt}_{j}")
                    for g in range(NPAIR):
                        nc.tensor.matmul(
                            v_ps,
                            xb_t[:, 2 * g:2 * g + 2, 128 * tt:128 * tt + 128],
                            wv_t[:, 2 * g:2 * g + 2, :],
                            start=(g == 0),
                            stop=(g == NPAIR - 1),
                            perf_mode=DR,
                        )
                    v1t = wpool.tile([128, LC], BF16, tag="v1t",
                                     name=f"v1t{tt}_{j}")
                    nc.sync.dma_start(
                        v1t, v1l[tq0 + 128 * tt:tq0 + 128 * tt + 128, :])
                    r = 4 * j + tt
                    nc.vector.tensor_add(vp[r // 2][:, r % 2, :], v_ps, v1t)

                # ---- attention per head (oproj of prev chunk interleaved) --
                attn2 = [
                    apool.tile([128, 2, TQ], FP8, tag=f"at{u}",
                               name=f"at{u}_{j}")
                    for u in range(2)
                ]
                npair = 2 * (j + 1)
                for h in range(4):
                    pv_ps = pspool.tile([128, TQ], F32, tag="pv", bufs=1,
                                        name=f"pv{h}_{j}")
                    se_ps = pspool.tile([128, TQ], F32, tag="se", bufs=1,
                                        name=f"se{h}_{j}")
                    for g in range(npair):
                        diag = (g >= npair - 2)
                        d0 = 256 if g == npair - 1 else 0
                        s2t = pspool.tile([128, 2, TQ], F32, tag="s2", bufs=2,
                                          name=f"s{h}_{g}_{j}")
                        for i in range(2):
                            ds = d0 + (128 if diag and i == 1 else 0)
                            nc.tensor.matmul(
                                s2t[:, i, ds:],
                                kT[h][:, 128 * (2 * g + i):
                                      128 * (2 * g + i) + 128],
                                qT[h][:, ds:],
                                start=True,
                                stop=True,
                            )
                        if diag:
                            nc.vector.tensor_add(
                                s2t[:, 0, d0:d0 + 128], s2t[:, 0, d0:d0 + 128],
                                mask_sb[:, 128:256])
                            nc.vector.tensor_add(
                                s2t[:, 1, d0:d0 + 256], s2t[:, 1, d0:d0 + 256],
                                mask_sb)
                        e2 = epool.tile([128, 2, TQ], FP8, tag="e",
                                        name=f"e{h}_{g}_{j}")
                        nc.scalar.activation(e2[:, :, d0:], s2t[:, :, d0:],
                                             AF.Exp, scale=SCALE)
                        nc.tensor.matmul(
                            pv_ps[:, d0:],
                            vp[g][:, :, 128 * h:128 * h + 128],
                            e2[:, :, d0:],
                            start=(g == 0),
                            stop=(g == npair - 1),
                            perf_mode=DR,
                        )
                        nc.tensor.matmul(
                            se_ps[:, d0:], ones8, e2[:, :, d0:],
                            start=(g == 0), stop=(g == npair - 1),
                            perf_mode=DR,
                        )
                    lnse = wpool.tile([128, TQ], F32, tag="lnse",
                                      name=f"lnse{h}_{j}")
                    nc.scalar.activation(lnse, se_ps, AF.Ln)
                    rec = wpool.tile([128, TQ], BF16, tag="rec",
                                     name=f"rec{h}_{j}")
                    nc.scalar.activation(rec, lnse, AF.Exp, scale=-1.0,
                                         bias=LN_A)
                    nc.vector.tensor_mul(attn2[h // 2][:, h % 2, :],
                                         pv_ps, rec)
                    if j > 0:
                        oproj(j - 1, attn_prev, 4 * h)
                attn_prev = attn2
            oproj(NCHUNK - 1, attn_prev, 0)
            oproj(NCHUNK - 1, attn_prev, 4)
            oproj(NCHUNK - 1, attn_prev, 8)
            oproj(NCHUNK - 1, attn_prev, 12)

    nc.finalize()
    return nc


def _host_prep(inputs):
    """Build the 8 per-core input maps (all numpy)."""
    x = np.asarray(inputs["x"], np.float32)
    v1 = np.asarray(inputs["v1"], np.float32)
    x_q = np.asarray(inputs["x_q"], np.float32)
    x_k = np.asarray(inputs["x_k"], np.float32)
    x_v = np.asarray(inputs["x_v"], np.float32)
    Wq = np.asarray(inputs["Wq"], np.float32)
    Wk = np.asarray(inputs["Wk"], np.float32)
    Wv = np.asarray(inputs["Wv"], np.float32)
    Wproj = np.asarray(inputs["Wproj"], np.float32)
    lamb = float(np.asarray(inputs["lamb"]))

    assert np.array_equal(x_q, x_k) and np.array_equal(x_q, x_v), (
        "kernel assumes shared token-shift mix vectors (x_q == x_k == x_v)"
    )

    def interleave(mat):
        # [C, F] -> [128, C//128, F] with [p, g, f] = mat[128*g + p, f]
        Cd, F = mat.shape
        return np.ascontiguousarray(
            mat.reshape(Cd // 128, 128, F).transpose(1, 0, 2))

    # token-shift blend, then transpose + interleave per batch
    sh = np.concatenate([np.zeros((B, 1, C), np.float32), x[:, :-1]], axis=1)
    xb = x * (1.0 - x_q) + sh * x_q
    xbI = [interleave(np.ascontiguousarray(xb[b_].T)).astype(_f8)
           for b_ in range(B)]

    # rope tables, duplicated halves; sin second half negated
    inv = 1.0 / (ROPE_THETA ** (np.arange(0, D, 2, dtype=np.float32) / D))
    fr = np.outer(np.arange(T, dtype=np.float32), inv)  # [T, 64]
    cosT = np.cos(fr).T.astype(np.float32)  # [64, T]
    sinT = np.sin(fr).T.astype(np.float32)
    cos2 = np.concatenate([cosT, cosT], axis=0).astype(_bf)
    sin2 = np.concatenate([sinT, -sinT], axis=0).astype(_bf)

    # causal mask strip [128, 256]: col c masked (=MASK_NEG) iff c < 128 + p
    p = np.arange(128)[:, None]
    c = np.arange(256)[None, :]
    mask256 = np.where(c < 128 + p, MASK_NEG, 0.0).astype(np.float32)
    permm = np.roll(np.eye(128, dtype=np.float32), 64, axis=0).astype(_bf)

    in_maps = []
    for cid in range(NCORES):
        b_ = cid // 4
        g_ = cid % 4
        L = slice(LC * g_, LC * g_ + LC)
        in_maps.append({
            "xbI": xbI[b_],
            "wqI": interleave(np.ascontiguousarray(
                (WSCALE * Wq[L]).T)).astype(_f8),
            "wkI": interleave(np.ascontiguousarray(
                (WSCALE * Wk[L]).T)).astype(_f8),
            "wvI": interleave(np.ascontiguousarray(
                (VSCALE * (1.0 - lamb) * Wv[L]).T)).astype(_f8),
            "wpI": interleave(np.ascontiguousarray(
                (WSCALE * Wproj[:, L]).T)).astype(_f8),
            "v1l": (VSCALE * lamb * v1[b_][:, L]).astype(_bf),
            "cos2": cos2,
            "sin2": sin2,
            "mask256": mask256,
            "perm": permm,
        })
    return in_maps


def _run(in_maps, trace=False):
    from concourse.bass_utils import run_bass_kernel_spmd

    if "nc" not in _prog_cache:
        _prog_cache["nc"] = _build_program()
    return run_bass_kernel_spmd(
        _prog_cache["nc"], in_maps, core_ids=list(range(NCORES)), trace=trace
    )


def kernel(**inputs) -> np.ndarray:
    residual = np.asarray(inputs["residual"], np.float32)
    in_maps = _host_prep(inputs)
    res = _run(in_maps)
    out = np.empty((B, T, C), np.float32)
    for b_ in range(B):
        acc = res.results[4 * b_]["outT"].astype(np.float32)
        for g_ in range(1, 4):
            acc = acc + res.results[4 * b_ + g_]["outT"]
        out[b_] = residual[b_] + acc.T * (1.0 / OUT_DIV)
    return out
